# revision 36
# baseline (speedup 1.0000x reference)
"""KSparseFFTClassifier Trainium2 kernel.

Math: reference computes
    h   = x @ W_proj.T + b_proj                      (bs, 129)
    h  *= scale  (sqrt(2) on dims 1..64)
    out = IDFT65(h[:, :65]) + h[:, 65:] @ Ws.T       (bs, 16384)

The zero-padded orthonormal IDFT of the 65 nonzero frequency components is a
dense matmul against a (65, N) cos/sin basis; the DC row of that basis is the
constant 1/sqrt(N).  So with M = [scaled cos/sin basis for h dims 1..64;
Ws.T]  (128 x N):

    out[b, n] = h[b, 1:129] @ M[:, n] + (h[b, 0] + 0) / sqrt(N)

i.e. a (bs,2048)x(2048,128) matmul, a (bs,128)x(128,N) matmul, and a
per-row scalar (the DC term) added during PSUM eviction.

Sharding: data-parallel over batch, 512 rows per core on 8 cores.
"""

import numpy as np

BS = 4096
IN_DIM = 2048
N = 16384
K = 32
SLACK = 64
NCORES = 8
BC = BS // NCORES        # 512 batch rows per core
P = 128
KT = IN_DIM // P         # 16 contraction tiles for matmul1
MCHUNK = 4096            # M load chunk (SBUF M tile free size)
MCH = N // MCHUNK        # 4
NCHUNK = 2048            # output store chunk (SBUF out tile free size)
NCH = N // NCHUNK        # 8

# matmul dtypes ("float32" = exact 2-pass fp32; "float32r" = single-pass
# fp32; "bfloat16" = half the HBM bytes; "float8e4" = quarter bytes).
# fp8 tensors are pre-scaled by 32 on the host (their raw values sit in
# e4m3's subnormal range) and the inverse scale is folded into the mm1
# eviction activation, so mm2's eviction stays a plain DC-add.  hT stays
# bf16 (mixed bf16 x fp8 matmuls run at full rate).
MM1_DT = "float8e4"          # x, w1t, w0
MM2_DT = "float8e4"          # combined IDFT-basis + Ws.T matrix
FP8_SCALE = 32.0
# fp8 DoubleRow perf mode: 2 contraction sub-tiles per matmul pass
USE_DR = True

_NC_CACHE = {}


def _scale_of(name):
    return FP8_SCALE if name == "float8e4" else 1.0


def _build_nc(mm1_name, mm2_name):
    import concourse.bacc as bacc
    import concourse.mybir as mybir
    import concourse.tile as tile

    f32 = mybir.dt.float32
    mm1 = getattr(mybir.dt, mm1_name)
    mm2 = getattr(mybir.dt, mm2_name)
    # hT feeds mm2 as lhsT; bf16 when M is narrow (mixed bf16 x fp8 matmuls
    # run at full PE rate -- but interleaving fp32 matmuls into the mm2
    # stream forces pipeline drains, so the dc transpose stays out of it)
    hdt = mybir.dt.bfloat16 if mm2_name in ("bfloat16", "float8e4") else mm2
    s1 = _scale_of(mm1_name)
    s2 = _scale_of(mm2_name)
    dr = USE_DR and mm1_name == "float8e4"
    DR = mybir.MatmulPerfMode.DoubleRow if dr else None

    nc = bacc.Bacc("TRN2", target_bir_lowering=False)

    xT = nc.dram_tensor("xT", [P, KT, BC], mm1, kind="ExternalInput")
    w1t = nc.dram_tensor("w1t", [P, KT, P], mm1, kind="ExternalInput")
    w0 = nc.dram_tensor("w0", [P, KT, 1], mm1, kind="ExternalInput")
    mmat = nc.dram_tensor("mmat", [P, N], mm2, kind="ExternalInput")
    # btc col 0: b_proj[1:129]/s2 bias; col 1: DC const b0/sqrt(N), all rows
    btc = nc.dram_tensor("btc", [P, 2], f32, kind="ExternalInput")
    out = nc.dram_tensor("out", [BC, N], f32, kind="ExternalOutput")

    Ident = mybir.ActivationFunctionType.Identity

    with tile.TileContext(nc) as tc:
        with (
            tc.tile_pool(name="wp", bufs=1) as wp,
            tc.tile_pool(name="xp", bufs=1) as xp,
            tc.tile_pool(name="mp", bufs=1) as mp,
            tc.tile_pool(name="hp", bufs=1) as hp,
            tc.tile_pool(name="op", bufs=4) as op,
            tc.tile_pool(name="ps", bufs=5, space="PSUM") as ps,
            tc.tile_pool(name="ps1", bufs=1, space="PSUM") as ps1,
            tc.tile_pool(name="ps2", bufs=1, space="PSUM") as ps2,
        ):
            # loads dispatch from the Sync engine in dependency order (each
            # dma_start costs ~650ns of serial dispatch there); stores also
            # go on Sync -- it is idle once the 8 load dispatches are done
            w1t_sb = wp.tile([P, KT, P], mm1, tag="w1t")
            nc.sync.dma_start(out=w1t_sb[:, :, :], in_=w1t[:, :, :])

            # x transposed, packed on host as k-major blocks of BC columns;
            # loaded in two halves so mm1 starts while the second half lands
            KH = KT // 2
            x_sb = xp.tile([P, KT, BC], mm1, tag="x")
            nc.sync.dma_start(out=x_sb[:, 0:KH, :], in_=xT[:, 0:KH, :])
            nc.sync.dma_start(out=x_sb[:, KH:KT, :], in_=xT[:, KH:KT, :])

            # tiny tensors next: the dc chain needs them right after mm1
            w0_sb = wp.tile([P, KT, 1], mm1, tag="w0")
            nc.sync.dma_start(out=w0_sb[:, :, :], in_=w0[:, :, :])
            btc_sb = wp.tile([P, 2], f32, tag="btc")
            nc.sync.dma_start(out=btc_sb[:, :], in_=btc[:, :])
            ones_sb = wp.tile([1, 1], f32, tag="ones")
            nc.vector.memset(ones_sb[:, :], 1.0)

            # combined IDFT-basis + Ws.T matrix, resident in SBUF
            mm = []
            for ti in range(MCH):
                m = mp.tile([P, MCHUNK], mm2, tag=f"m{ti}")
                nc.sync.dma_start(
                    out=m[:, :], in_=mmat[:, ti * MCHUNK:(ti + 1) * MCHUNK]
                )
                mm.append(m)

            # matmul1: hT[d, b] for d = h dims 1..128 (fp8 DoubleRow: two
            # contraction sub-tiles per pass)
            kstep = 2 if dr else 1
            hT_ps = ps1.tile([P, BC], f32, tag="hT")
            for kt in range(0, KT, kstep):
                nc.tensor.matmul(
                    hT_ps[:, :],
                    lhsT=w1t_sb[:, kt:kt + kstep, :],
                    rhs=x_sb[:, kt:kt + kstep, :],
                    start=(kt == 0),
                    stop=(kt + kstep == KT),
                    perf_mode=DR,
                )
            # evict h/(s1*s2): 1/s1 undoes the host fp8 pre-scale of w1t, the
            # extra 1/s2 cancels against M's host pre-scale in mm2.  btc col0
            # is pre-divided by s2 on the host.
            hT_sb = hp.tile([P, BC], hdt, tag="hT_sb")
            nc.scalar.activation(
                hT_sb[:, :], hT_ps[:, :], Ident,
                bias=btc_sb[:, 0:1], scale=float(1.0 / (s1 * s2)),
            )

            # first mm2 group runs on the PE before the dcr chain; its PSUM
            # tiles wait in banks and evict the moment dc_sb is ready, so the
            # store stream starts ~4us earlier
            g1 = []
            for s in range(NCHUNK // 512):
                pt = ps.tile([P, 512], f32, tag="mm2")
                nc.tensor.matmul(
                    pt[:, :],
                    lhsT=hT_sb[:, 0:P],
                    rhs=mm[0][:, s * 512:(s + 1) * 512],
                    start=True,
                    stop=True,
                )
                g1.append(pt)

            # dc row: h dim 0 as (1, BC), scaled to the final per-batch
            # output bias, then PE-transposed to (P, BC//P)
            dcr_ps = ps2.tile([1, BC], f32, tag="dcr")
            for kt in range(KT):
                nc.tensor.matmul(
                    dcr_ps[:, :],
                    lhsT=w0_sb[:, kt:kt + 1, :],
                    rhs=x_sb[:, kt:kt + 1, :],
                    start=(kt == 0),
                    stop=(kt == KT - 1),
                )
            dcr_sb = hp.tile([1, BC], f32, tag="dcr_sb")
            nc.scalar.activation(
                dcr_sb[:, :], dcr_ps[:, :], Ident,
                bias=btc_sb[0:1, 1:2], scale=float(1.0 / (s1 * np.sqrt(N))),
            )
            dc_sb = hp.tile([P, BC // P], f32, tag="dc_sb")
            for j in range(BC // P):
                dcc_ps = ps2.tile([P, 1], f32, tag="dcc")
                nc.tensor.matmul(
                    dcc_ps[:, :],
                    lhsT=dcr_sb[0:1, j * P:(j + 1) * P],
                    rhs=ones_sb[0:1, 0:1],
                    start=True,
                    stop=True,
                )
                nc.scalar.copy(dc_sb[:, j:j + 1], dcc_ps[:, :])

            # matmul2 + DC bias-add eviction (split scalar/vector) + store
            for ti in range(NCH):
                mt = mm[ti * NCHUNK // MCHUNK]
                mo = (ti * NCHUNK) % MCHUNK
                for j in range(BC // P):
                    ob = op.tile([P, NCHUNK], f32, tag="ob")
                    for s in range(NCHUNK // 512):
                        if ti == 0 and j == 0:
                            pt = g1[s]
                        else:
                            pt = ps.tile([P, 512], f32, tag="mm2")
                            nc.tensor.matmul(
                                pt[:, :],
                                lhsT=hT_sb[:, j * P:(j + 1) * P],
                                rhs=mt[:, mo + s * 512:mo + (s + 1) * 512],
                                start=True,
                                stop=True,
                            )
                        dst = ob[:, s * 512:(s + 1) * 512]
                        if s % 2 == 0:
                            nc.vector.tensor_scalar_add(
                                dst, pt[:, :], dc_sb[:, j:j + 1]
                            )
                        else:
                            nc.scalar.activation(
                                dst, pt[:, :], Ident, bias=dc_sb[:, j:j + 1],
                            )
                    nc.sync.dma_start(
                        out=out[j * P:(j + 1) * P, ti * NCHUNK:(ti + 1) * NCHUNK],
                        in_=ob[:, :],
                    )
    nc.compile()
    return nc


def _get_nc():
    key = (MM1_DT, MM2_DT, USE_DR)
    if key not in _NC_CACHE:
        _NC_CACHE[key] = _build_nc(MM1_DT, MM2_DT)
    return _NC_CACHE[key]


def _np_dt(name):
    if name.startswith("float32"):
        return np.float32
    import ml_dtypes
    if name == "float8e4":
        return np.dtype(ml_dtypes.float8_e4m3fn)
    return np.dtype(getattr(ml_dtypes, name))


def _host_pack(x, W_proj, b_proj, Ws):
    dt1 = _np_dt(MM1_DT)
    dt2 = _np_dt(MM2_DT)
    s1 = _scale_of(MM1_DT)
    s2 = _scale_of(MM2_DT)
    SQRT2 = np.float64(np.sqrt(np.float32(2.0)))
    n_idx = np.arange(N, dtype=np.float64)
    k_idx = np.arange(1, K + 1, dtype=np.float64)
    theta = (2.0 * np.pi / N) * np.outer(k_idx, n_idx)
    M = np.empty((P, N), np.float32)
    isqn = 1.0 / np.sqrt(np.float64(N))
    M[0:2 * K:2] = (SQRT2 * isqn) * np.cos(theta)
    M[1:2 * K:2] = (SQRT2 * isqn) * np.sin(theta)
    M[2 * K:] = Ws.T
    M = np.ascontiguousarray(M * np.float32(s2), dt2)

    w1 = W_proj[1:P + 1] * np.float32(s1)                 # (128, 2048)
    w1t = np.ascontiguousarray(
        w1.T.reshape(KT, P, P).transpose(1, 0, 2), dt1
    )                                                     # (128, 16, 128)
    w0 = np.ascontiguousarray(
        W_proj[0].reshape(KT, P).T * np.float32(s1), dt1
    ).reshape(P, KT, 1)                                   # (128, 16, 1)
    btc = np.zeros((P, 2), np.float32)
    btc[:, 0] = b_proj[1:P + 1] / np.float32(s2)
    btc[:, 1] = np.float32(np.float64(b_proj[0]) / np.sqrt(np.float64(N)))

    xts = []
    for c in range(NCORES):
        xc = x[c * BC:(c + 1) * BC]                        # (512, 2048)
        xt = np.ascontiguousarray(
            xc.T.reshape(KT, P, BC).transpose(1, 0, 2), dt1
        )                                                 # (128, 16, 512)
        xts.append(xt)
    return M, w1t, w0, btc, xts


def kernel(x, W_proj, b_proj, Ws, _trace=False, _tmpdir=None):
    from concourse import bass_utils

    x = np.ascontiguousarray(x, np.float32)
    W_proj = np.ascontiguousarray(W_proj, np.float32)
    b_proj = np.ascontiguousarray(b_proj, np.float32)
    Ws = np.ascontiguousarray(Ws, np.float32)

    M, w1t, w0, btc, xts = _host_pack(x, W_proj, b_proj, Ws)
    nc = _get_nc()

    in_maps = [
        {"xT": xts[c], "w1t": w1t, "w0": w0, "mmat": M, "btc": btc}
        for c in range(NCORES)
    ]
    kw = {}
    if _trace:
        kw = dict(trace=True, tmpdir=_tmpdir, trace_cores=[0])
    res = bass_utils.run_bass_kernel_spmd(nc, in_maps, core_ids=list(range(NCORES)), **kw)
    out = np.concatenate([r["out"] for r in res.results], axis=0)
    if _trace:
        return out, res
    return out



# revision 40
# speedup vs baseline: 1.0704x; 1.0704x over previous
"""KSparseFFTClassifier Trainium2 kernel.

Math: reference computes
    h   = x @ W_proj.T + b_proj                      (bs, 129)
    h  *= scale  (sqrt(2) on dims 1..64)
    out = IDFT65(h[:, :65]) + h[:, 65:] @ Ws.T       (bs, 16384)

The zero-padded orthonormal IDFT of the 65 nonzero frequency components is a
dense matmul against a (65, N) cos/sin basis; the DC row of that basis is the
constant 1/sqrt(N).  So with M = [scaled cos/sin basis for h dims 1..64;
Ws.T]  (128 x N):

    out[b, n] = h[b, 1:129] @ M[:, n] + (h[b, 0] + 0) / sqrt(N)

i.e. a (bs,2048)x(2048,128) matmul, a (bs,128)x(128,N) matmul, and a
per-row scalar (the DC term) added during PSUM eviction.

Sharding: data-parallel over batch, 512 rows per core on 8 cores.
"""

import numpy as np

BS = 4096
IN_DIM = 2048
N = 16384
K = 32
SLACK = 64
NCORES = 8
BC = BS // NCORES        # 512 batch rows per core
P = 128
KT = IN_DIM // P         # 16 contraction tiles for matmul1
MCHUNK = 4096            # M load chunk (SBUF M tile free size)
MCH = N // MCHUNK        # 4
NCHUNK = 2048            # output store chunk (SBUF out tile free size)
NCH = N // NCHUNK        # 8

# matmul dtypes ("float32" = exact 2-pass fp32; "float32r" = single-pass
# fp32; "bfloat16" = half the HBM bytes; "float8e4" = quarter bytes).
# fp8 tensors are pre-scaled by 32 on the host (their raw values sit in
# e4m3's subnormal range) and the inverse scale is folded into the mm1
# eviction activation, so mm2's eviction stays a plain DC-add.  hT stays
# bf16 (mixed bf16 x fp8 matmuls run at full rate).
MM1_DT = "float8e4"          # x, w1t, w0
MM2_DT = "float8e4"          # combined IDFT-basis + Ws.T matrix
FP8_SCALE = 32.0
# fp8 DoubleRow perf mode: 2 contraction sub-tiles per matmul pass
USE_DR = True

_NC_CACHE = {}


def _scale_of(name):
    return FP8_SCALE if name == "float8e4" else 1.0


def _build_nc(mm1_name, mm2_name):
    import concourse.bacc as bacc
    import concourse.mybir as mybir
    import concourse.tile as tile

    f32 = mybir.dt.float32
    mm1 = getattr(mybir.dt, mm1_name)
    mm2 = getattr(mybir.dt, mm2_name)
    # hT feeds mm2 as lhsT; bf16 when M is narrow (mixed bf16 x fp8 matmuls
    # run at full PE rate -- but interleaving fp32 matmuls into the mm2
    # stream forces pipeline drains, so the dc transpose stays out of it)
    hdt = mybir.dt.bfloat16 if mm2_name in ("bfloat16", "float8e4") else mm2
    s1 = _scale_of(mm1_name)
    s2 = _scale_of(mm2_name)
    dr = USE_DR and mm1_name == "float8e4"
    DR = mybir.MatmulPerfMode.DoubleRow if dr else None

    nc = bacc.Bacc("TRN2", target_bir_lowering=False)

    xT = nc.dram_tensor("xT", [P, KT, BC], mm1, kind="ExternalInput")
    w1t = nc.dram_tensor("w1t", [P, KT, P], mm1, kind="ExternalInput")
    w0 = nc.dram_tensor("w0", [P, KT, 1], mm1, kind="ExternalInput")
    mmat = nc.dram_tensor("mmat", [P, N], mm2, kind="ExternalInput")
    # btc col 0: b_proj[1:129]/s2 bias; col 1: DC const b0/sqrt(N), all rows
    btc = nc.dram_tensor("btc", [P, 2], f32, kind="ExternalInput")
    out = nc.dram_tensor("out", [BC, N], f32, kind="ExternalOutput")

    Ident = mybir.ActivationFunctionType.Identity

    with tile.TileContext(nc) as tc:
        with (
            tc.tile_pool(name="wp", bufs=1) as wp,
            tc.tile_pool(name="xp", bufs=1) as xp,
            tc.tile_pool(name="mp", bufs=1) as mp,
            tc.tile_pool(name="hp", bufs=1) as hp,
            tc.tile_pool(name="op", bufs=4) as op,
            tc.tile_pool(name="ps", bufs=5, space="PSUM") as ps,
            tc.tile_pool(name="ps1", bufs=1, space="PSUM") as ps1,
            tc.tile_pool(name="ps2", bufs=1, space="PSUM") as ps2,
        ):
            # loads dispatch from the Sync engine in dependency order (each
            # dma_start costs ~650ns of serial dispatch there); stores also
            # go on Sync -- it is idle once the 8 load dispatches are done
            w1t_sb = wp.tile([P, KT, P], mm1, tag="w1t")
            nc.sync.dma_start(out=w1t_sb[:, :, :], in_=w1t[:, :, :])

            # x transposed, packed on host as k-major blocks of BC columns;
            # loaded in two halves so mm1 starts while the second half lands
            KH = KT // 2
            x_sb = xp.tile([P, KT, BC], mm1, tag="x")
            nc.sync.dma_start(out=x_sb[:, 0:KH, :], in_=xT[:, 0:KH, :])
            nc.sync.dma_start(out=x_sb[:, KH:KT, :], in_=xT[:, KH:KT, :])

            # tiny tensors next: the dc chain needs them right after mm1
            w0_sb = wp.tile([P, KT, 1], mm1, tag="w0")
            nc.sync.dma_start(out=w0_sb[:, :, :], in_=w0[:, :, :])
            btc_sb = wp.tile([P, 2], f32, tag="btc")
            nc.sync.dma_start(out=btc_sb[:, :], in_=btc[:, :])
            ones_sb = wp.tile([1, 1], mybir.dt.bfloat16, tag="ones")
            nc.vector.memset(ones_sb[:, :], 1.0)

            # combined IDFT-basis + Ws.T matrix, resident in SBUF
            mm = []
            for ti in range(MCH):
                m = mp.tile([P, MCHUNK], mm2, tag=f"m{ti}")
                nc.sync.dma_start(
                    out=m[:, :], in_=mmat[:, ti * MCHUNK:(ti + 1) * MCHUNK]
                )
                mm.append(m)

            # matmul1: hT[d, b] for d = h dims 1..128 (fp8 DoubleRow: two
            # contraction sub-tiles per pass)
            kstep = 2 if dr else 1
            hT_ps = ps1.tile([P, BC], f32, tag="hT")
            for kt in range(0, KT, kstep):
                nc.tensor.matmul(
                    hT_ps[:, :],
                    lhsT=w1t_sb[:, kt:kt + kstep, :],
                    rhs=x_sb[:, kt:kt + kstep, :],
                    start=(kt == 0),
                    stop=(kt + kstep == KT),
                    perf_mode=DR,
                )
            # evict h/(s1*s2): 1/s1 undoes the host fp8 pre-scale of w1t, the
            # extra 1/s2 cancels against M's host pre-scale in mm2.  btc col0
            # is pre-divided by s2 on the host.
            hT_sb = hp.tile([P, BC], hdt, tag="hT_sb")
            nc.scalar.activation(
                hT_sb[:, :], hT_ps[:, :], Ident,
                bias=btc_sb[:, 0:1], scale=float(1.0 / (s1 * s2)),
            )

            # dc row: h dim 0 as (1, BC), scaled to the final per-batch
            # output bias, then PE-transposed to (P, BC//P)
            dcr_ps = ps2.tile([1, BC], f32, tag="dcr")
            for kt in range(KT):
                nc.tensor.matmul(
                    dcr_ps[:, :],
                    lhsT=w0_sb[:, kt:kt + 1, :],
                    rhs=x_sb[:, kt:kt + 1, :],
                    start=(kt == 0),
                    stop=(kt == KT - 1),
                )
            # bf16 here: an fp32 lhsT in the transpose matmuls would force
            # LOW/HIGH ldweights pairs into the PE stream (costs ~1.4e-3 rel)
            dcr_sb = hp.tile([1, BC], mybir.dt.bfloat16, tag="dcr_sb")
            nc.scalar.activation(
                dcr_sb[:, :], dcr_ps[:, :], Ident,
                bias=btc_sb[0:1, 1:2], scale=float(1.0 / (s1 * np.sqrt(N))),
            )
            dc_sb = hp.tile([P, BC // P], f32, tag="dc_sb")
            for j in range(BC // P):
                dcc_ps = ps2.tile([P, 1], f32, tag="dcc")
                nc.tensor.matmul(
                    dcc_ps[:, :],
                    lhsT=dcr_sb[0:1, j * P:(j + 1) * P],
                    rhs=ones_sb[0:1, 0:1],
                    start=True,
                    stop=True,
                )
                nc.scalar.copy(dc_sb[:, j:j + 1], dcc_ps[:, :])

            # matmul2 + DC bias-add eviction (split scalar/vector) + store
            for ti in range(NCH):
                mt = mm[ti * NCHUNK // MCHUNK]
                mo = (ti * NCHUNK) % MCHUNK
                for j in range(BC // P):
                    ob = op.tile([P, NCHUNK], f32, tag="ob")
                    for s in range(NCHUNK // 512):
                        pt = ps.tile([P, 512], f32, tag="mm2")
                        nc.tensor.matmul(
                            pt[:, :],
                            lhsT=hT_sb[:, j * P:(j + 1) * P],
                            rhs=mt[:, mo + s * 512:mo + (s + 1) * 512],
                            start=True,
                            stop=True,
                        )
                        dst = ob[:, s * 512:(s + 1) * 512]
                        if s % 2 == 0:
                            nc.vector.tensor_scalar_add(
                                dst, pt[:, :], dc_sb[:, j:j + 1]
                            )
                        else:
                            nc.scalar.activation(
                                dst, pt[:, :], Ident, bias=dc_sb[:, j:j + 1],
                            )
                    nc.sync.dma_start(
                        out=out[j * P:(j + 1) * P, ti * NCHUNK:(ti + 1) * NCHUNK],
                        in_=ob[:, :],
                    )
    nc.compile()
    return nc


def _get_nc():
    key = (MM1_DT, MM2_DT, USE_DR)
    if key not in _NC_CACHE:
        _NC_CACHE[key] = _build_nc(MM1_DT, MM2_DT)
    return _NC_CACHE[key]


def _np_dt(name):
    if name.startswith("float32"):
        return np.float32
    import ml_dtypes
    if name == "float8e4":
        return np.dtype(ml_dtypes.float8_e4m3fn)
    return np.dtype(getattr(ml_dtypes, name))


def _host_pack(x, W_proj, b_proj, Ws):
    dt1 = _np_dt(MM1_DT)
    dt2 = _np_dt(MM2_DT)
    s1 = _scale_of(MM1_DT)
    s2 = _scale_of(MM2_DT)
    SQRT2 = np.float64(np.sqrt(np.float32(2.0)))
    n_idx = np.arange(N, dtype=np.float64)
    k_idx = np.arange(1, K + 1, dtype=np.float64)
    theta = (2.0 * np.pi / N) * np.outer(k_idx, n_idx)
    M = np.empty((P, N), np.float32)
    isqn = 1.0 / np.sqrt(np.float64(N))
    M[0:2 * K:2] = (SQRT2 * isqn) * np.cos(theta)
    M[1:2 * K:2] = (SQRT2 * isqn) * np.sin(theta)
    M[2 * K:] = Ws.T
    M = np.ascontiguousarray(M * np.float32(s2), dt2)

    w1 = W_proj[1:P + 1] * np.float32(s1)                 # (128, 2048)
    w1t = np.ascontiguousarray(
        w1.T.reshape(KT, P, P).transpose(1, 0, 2), dt1
    )                                                     # (128, 16, 128)
    w0 = np.ascontiguousarray(
        W_proj[0].reshape(KT, P).T * np.float32(s1), dt1
    ).reshape(P, KT, 1)                                   # (128, 16, 1)
    btc = np.zeros((P, 2), np.float32)
    btc[:, 0] = b_proj[1:P + 1] / np.float32(s2)
    btc[:, 1] = np.float32(np.float64(b_proj[0]) / np.sqrt(np.float64(N)))

    xts = []
    for c in range(NCORES):
        xc = x[c * BC:(c + 1) * BC]                        # (512, 2048)
        xt = np.ascontiguousarray(
            xc.T.reshape(KT, P, BC).transpose(1, 0, 2), dt1
        )                                                 # (128, 16, 512)
        xts.append(xt)
    return M, w1t, w0, btc, xts


def kernel(x, W_proj, b_proj, Ws, _trace=False, _tmpdir=None):
    from concourse import bass_utils

    x = np.ascontiguousarray(x, np.float32)
    W_proj = np.ascontiguousarray(W_proj, np.float32)
    b_proj = np.ascontiguousarray(b_proj, np.float32)
    Ws = np.ascontiguousarray(Ws, np.float32)

    M, w1t, w0, btc, xts = _host_pack(x, W_proj, b_proj, Ws)
    nc = _get_nc()

    in_maps = [
        {"xT": xts[c], "w1t": w1t, "w0": w0, "mmat": M, "btc": btc}
        for c in range(NCORES)
    ]
    kw = {}
    if _trace:
        kw = dict(trace=True, tmpdir=_tmpdir, trace_cores=[0])
    res = bass_utils.run_bass_kernel_spmd(nc, in_maps, core_ids=list(range(NCORES)), **kw)
    out = np.concatenate([r["out"] for r in res.results], axis=0)
    if _trace:
        return out, res
    return out



# revision 44
# speedup vs baseline: 1.0709x; 1.0004x over previous
"""KSparseFFTClassifier Trainium2 kernel.

Math: reference computes
    h   = x @ W_proj.T + b_proj                      (bs, 129)
    h  *= scale  (sqrt(2) on dims 1..64)
    out = IDFT65(h[:, :65]) + h[:, 65:] @ Ws.T       (bs, 16384)

The zero-padded orthonormal IDFT of the 65 nonzero frequency components is a
dense matmul against a (65, N) cos/sin basis; the DC row of that basis is the
constant 1/sqrt(N).  So with M = [scaled cos/sin basis for h dims 1..64;
Ws.T]  (128 x N):

    out[b, n] = h[b, 1:129] @ M[:, n] + (h[b, 0] + 0) / sqrt(N)

i.e. a (bs,2048)x(2048,128) matmul, a (bs,128)x(128,N) matmul, and a
per-row scalar (the DC term) added during PSUM eviction.

Sharding: data-parallel over batch, 512 rows per core on 8 cores.
"""

import numpy as np

BS = 4096
IN_DIM = 2048
N = 16384
K = 32
SLACK = 64
NCORES = 8
BC = BS // NCORES        # 512 batch rows per core
P = 128
KT = IN_DIM // P         # 16 contraction tiles for matmul1
MCHUNK = 4096            # M load chunk (SBUF M tile free size)
MCH = N // MCHUNK        # 4
NCHUNK = 2048            # output store chunk (SBUF out tile free size)
NCH = N // NCHUNK        # 8

# matmul dtypes ("float32" = exact 2-pass fp32; "float32r" = single-pass
# fp32; "bfloat16" = half the HBM bytes; "float8e4" = quarter bytes).
# fp8 tensors are pre-scaled by 32 on the host (their raw values sit in
# e4m3's subnormal range) and the inverse scale is folded into the mm1
# eviction activation, so mm2's eviction stays a plain DC-add.  hT stays
# bf16 (mixed bf16 x fp8 matmuls run at full rate).
MM1_DT = "float8e4"          # x, w1t, w0
MM2_DT = "float8e4"          # combined IDFT-basis + Ws.T matrix
FP8_SCALE = 32.0
# fp8 DoubleRow perf mode: 2 contraction sub-tiles per matmul pass
USE_DR = True

_NC_CACHE = {}


def _scale_of(name):
    return FP8_SCALE if name == "float8e4" else 1.0


def _build_nc(mm1_name, mm2_name):
    import concourse.bacc as bacc
    import concourse.mybir as mybir
    import concourse.tile as tile

    f32 = mybir.dt.float32
    mm1 = getattr(mybir.dt, mm1_name)
    mm2 = getattr(mybir.dt, mm2_name)
    # hT feeds mm2 as lhsT; bf16 when M is narrow (mixed bf16 x fp8 matmuls
    # run at full PE rate -- but interleaving fp32 matmuls into the mm2
    # stream forces pipeline drains, so the dc transpose stays out of it)
    hdt = mybir.dt.bfloat16 if mm2_name in ("bfloat16", "float8e4") else mm2
    s1 = _scale_of(mm1_name)
    s2 = _scale_of(mm2_name)
    dr = USE_DR and mm1_name == "float8e4"
    DR = mybir.MatmulPerfMode.DoubleRow if dr else None

    nc = bacc.Bacc("TRN2", target_bir_lowering=False)

    xT = nc.dram_tensor("xT", [P, KT, BC], mm1, kind="ExternalInput")
    w1t = nc.dram_tensor("w1t", [P, KT, P], mm1, kind="ExternalInput")
    w0 = nc.dram_tensor("w0", [P, KT, 2], mm1, kind="ExternalInput")
    mmat = nc.dram_tensor("mmat", [P, N], mm2, kind="ExternalInput")
    # btc col 0: b_proj[1:129]/s2 bias; col 1: DC const b0/sqrt(N), all rows
    btc = nc.dram_tensor("btc", [P, 2], f32, kind="ExternalInput")
    out = nc.dram_tensor("out", [BC, N], f32, kind="ExternalOutput")

    Ident = mybir.ActivationFunctionType.Identity

    with tile.TileContext(nc) as tc:
        with (
            tc.tile_pool(name="wp", bufs=1) as wp,
            tc.tile_pool(name="xp", bufs=1) as xp,
            tc.tile_pool(name="mp", bufs=1) as mp,
            tc.tile_pool(name="hp", bufs=1) as hp,
            tc.tile_pool(name="op", bufs=4) as op,
            tc.tile_pool(name="ps", bufs=5, space="PSUM") as ps,
            tc.tile_pool(name="ps1", bufs=1, space="PSUM") as ps1,
            tc.tile_pool(name="ps2", bufs=1, space="PSUM") as ps2,
        ):
            # loads dispatch from the Sync engine in dependency order (each
            # dma_start costs ~650ns of serial dispatch there); stores also
            # go on Sync -- it is idle once the load dispatches are done
            w1t_sb = wp.tile([P, KT, P], mm1, tag="w1t")
            nc.sync.dma_start(out=w1t_sb[:, :, :], in_=w1t[:, :, :])
            # x in one DMA: 8KB contiguous rows hit the full per-queue rate
            x_sb = xp.tile([P, KT, BC], mm1, tag="x")
            nc.sync.dma_start(out=x_sb[:, :, :], in_=xT[:, :, :])

            # tiny tensors next: the dc chain needs them right after mm1
            w0_sb = wp.tile([P, KT, 2], mm1, tag="w0")
            nc.sync.dma_start(out=w0_sb[:, :, :], in_=w0[:, :, :])
            btc_sb = wp.tile([P, 2], f32, tag="btc")
            nc.sync.dma_start(out=btc_sb[:, :], in_=btc[:, :])
            ones_sb = wp.tile([1, 1], mybir.dt.bfloat16, tag="ones")
            nc.vector.memset(ones_sb[:, :], 1.0)

            # combined IDFT-basis + Ws.T matrix, resident in SBUF
            mm = []
            for ti in range(MCH):
                m = mp.tile([P, MCHUNK], mm2, tag=f"m{ti}")
                nc.sync.dma_start(
                    out=m[:, :], in_=mmat[:, ti * MCHUNK:(ti + 1) * MCHUNK]
                )
                mm.append(m)

            # matmul1: hT[d, b] for d = h dims 1..128 (fp8 DoubleRow: two
            # contraction sub-tiles per pass)
            kstep = 2 if dr else 1
            hT_ps = ps1.tile([P, BC], f32, tag="hT")
            for kt in range(0, KT, kstep):
                nc.tensor.matmul(
                    hT_ps[:, :],
                    lhsT=w1t_sb[:, kt:kt + kstep, :],
                    rhs=x_sb[:, kt:kt + kstep, :],
                    start=(kt == 0),
                    stop=(kt + kstep == KT),
                    perf_mode=DR,
                )
            # evict h/(s1*s2): 1/s1 undoes the host fp8 pre-scale of w1t, the
            # extra 1/s2 cancels against M's host pre-scale in mm2.  btc col0
            # is pre-divided by s2 on the host.
            hT_sb = hp.tile([P, BC], hdt, tag="hT_sb")
            nc.scalar.activation(
                hT_sb[:, :], hT_ps[:, :], Ident,
                bias=btc_sb[:, 0:1], scale=float(1.0 / (s1 * s2)),
            )

            # dc row: h dim 0 as (1, BC), scaled to the final per-batch
            # output bias, then PE-transposed to (P, BC//P)
            dcr_ps = ps2.tile([1, BC], f32, tag="dcr")
            for kt in range(KT):
                nc.tensor.matmul(
                    dcr_ps[:, :],
                    lhsT=w0_sb[:, kt:kt + 1, 0:1],
                    rhs=x_sb[:, kt:kt + 1, :],
                    start=(kt == 0),
                    stop=(kt == KT - 1),
                )
            # bf16 here: an fp32 lhsT in the transpose matmuls would force
            # LOW/HIGH ldweights pairs into the PE stream (costs ~1.4e-3 rel)
            dcr_sb = hp.tile([1, BC], mybir.dt.bfloat16, tag="dcr_sb")
            nc.scalar.activation(
                dcr_sb[:, :], dcr_ps[0:1, :], Ident,
                bias=btc_sb[0:1, 1:2], scale=float(1.0 / (s1 * np.sqrt(N))),
            )
            dc_sb = hp.tile([P, BC // P], f32, tag="dc_sb")
            for j in range(BC // P):
                dcc_ps = ps2.tile([P, 1], f32, tag="dcc")
                nc.tensor.matmul(
                    dcc_ps[:, :],
                    lhsT=dcr_sb[0:1, j * P:(j + 1) * P],
                    rhs=ones_sb[0:1, 0:1],
                    start=True,
                    stop=True,
                )
                nc.scalar.copy(dc_sb[:, j:j + 1], dcc_ps[:, :])

            # matmul2 + DC bias-add eviction (split scalar/vector) + store
            for ti in range(NCH):
                mt = mm[ti * NCHUNK // MCHUNK]
                mo = (ti * NCHUNK) % MCHUNK
                for j in range(BC // P):
                    ob = op.tile([P, NCHUNK], f32, tag="ob")
                    for s in range(NCHUNK // 512):
                        pt = ps.tile([P, 512], f32, tag="mm2")
                        nc.tensor.matmul(
                            pt[:, :],
                            lhsT=hT_sb[:, j * P:(j + 1) * P],
                            rhs=mt[:, mo + s * 512:mo + (s + 1) * 512],
                            start=True,
                            stop=True,
                        )
                        dst = ob[:, s * 512:(s + 1) * 512]
                        if s % 2 == 0:
                            nc.vector.tensor_scalar_add(
                                dst, pt[:, :], dc_sb[:, j:j + 1]
                            )
                        else:
                            nc.scalar.activation(
                                dst, pt[:, :], Ident, bias=dc_sb[:, j:j + 1],
                            )
                    nc.sync.dma_start(
                        out=out[j * P:(j + 1) * P, ti * NCHUNK:(ti + 1) * NCHUNK],
                        in_=ob[:, :],
                    )
    nc.compile()
    return nc


def _get_nc():
    key = (MM1_DT, MM2_DT, USE_DR)
    if key not in _NC_CACHE:
        _NC_CACHE[key] = _build_nc(MM1_DT, MM2_DT)
    return _NC_CACHE[key]


def _np_dt(name):
    if name.startswith("float32"):
        return np.float32
    import ml_dtypes
    if name == "float8e4":
        return np.dtype(ml_dtypes.float8_e4m3fn)
    return np.dtype(getattr(ml_dtypes, name))


def _host_pack(x, W_proj, b_proj, Ws):
    dt1 = _np_dt(MM1_DT)
    dt2 = _np_dt(MM2_DT)
    s1 = _scale_of(MM1_DT)
    s2 = _scale_of(MM2_DT)
    SQRT2 = np.float64(np.sqrt(np.float32(2.0)))
    n_idx = np.arange(N, dtype=np.float64)
    k_idx = np.arange(1, K + 1, dtype=np.float64)
    theta = (2.0 * np.pi / N) * np.outer(k_idx, n_idx)
    M = np.empty((P, N), np.float32)
    isqn = 1.0 / np.sqrt(np.float64(N))
    M[0:2 * K:2] = (SQRT2 * isqn) * np.cos(theta)
    M[1:2 * K:2] = (SQRT2 * isqn) * np.sin(theta)
    M[2 * K:] = Ws.T
    M = np.ascontiguousarray(M * np.float32(s2), dt2)

    w1 = W_proj[1:P + 1] * np.float32(s1)                 # (128, 2048)
    w1t = w1.T.reshape(KT, P, P).transpose(1, 0, 2)       # (128, 16, 128)
    w0 = np.ascontiguousarray(
        np.repeat(
            (W_proj[0].reshape(KT, P).T * np.float32(s1))[:, :, None], 2, axis=2
        ),
        dt1,
    )                                                     # (128, 16, 2)
    btc = np.zeros((P, 2), np.float32)
    btc[:, 0] = b_proj[1:P + 1] / np.float32(s2)
    btc[:, 1] = np.float32(np.float64(b_proj[0]) / np.sqrt(np.float64(N)))

    w1t = np.ascontiguousarray(w1t, dt1)
    xts = []
    for c in range(NCORES):
        xc = x[c * BC:(c + 1) * BC]                        # (512, 2048)
        xt = np.ascontiguousarray(
            xc.T.reshape(KT, P, BC).transpose(1, 0, 2), dt1
        )                                                 # (128, 16, 512)
        xts.append(xt)
    return M, w1t, w0, btc, xts


def kernel(x, W_proj, b_proj, Ws, _trace=False, _tmpdir=None):
    from concourse import bass_utils

    x = np.ascontiguousarray(x, np.float32)
    W_proj = np.ascontiguousarray(W_proj, np.float32)
    b_proj = np.ascontiguousarray(b_proj, np.float32)
    Ws = np.ascontiguousarray(Ws, np.float32)

    M, w1t, w0, btc, xts = _host_pack(x, W_proj, b_proj, Ws)
    nc = _get_nc()

    in_maps = [
        {"xT": xts[c], "w1t": w1t, "w0": w0, "mmat": M, "btc": btc}
        for c in range(NCORES)
    ]
    kw = {}
    if _trace:
        kw = dict(trace=True, tmpdir=_tmpdir, trace_cores=[0])
    res = bass_utils.run_bass_kernel_spmd(nc, in_maps, core_ids=list(range(NCORES)), **kw)
    out = np.concatenate([r["out"] for r in res.results], axis=0)
    if _trace:
        return out, res
    return out



# revision 46
# speedup vs baseline: 1.0938x; 1.0214x over previous
"""KSparseFFTClassifier Trainium2 kernel.

Math: reference computes
    h   = x @ W_proj.T + b_proj                      (bs, 129)
    h  *= scale  (sqrt(2) on dims 1..64)
    out = IDFT65(h[:, :65]) + h[:, 65:] @ Ws.T       (bs, 16384)

The zero-padded orthonormal IDFT of the 65 nonzero frequency components is a
dense matmul against a (65, N) cos/sin basis; the DC row of that basis is the
constant 1/sqrt(N).  So with M = [scaled cos/sin basis for h dims 1..64;
Ws.T]  (128 x N):

    out[b, n] = h[b, 1:129] @ M[:, n] + (h[b, 0] + 0) / sqrt(N)

i.e. a (bs,2048)x(2048,128) matmul, a (bs,128)x(128,N) matmul, and a
per-row scalar (the DC term) added during PSUM eviction.

Sharding: data-parallel over batch, 512 rows per core on 8 cores.
"""

import numpy as np

BS = 4096
IN_DIM = 2048
N = 16384
K = 32
SLACK = 64
NCORES = 8
BC = BS // NCORES        # 512 batch rows per core
P = 128
KT = IN_DIM // P         # 16 contraction tiles for matmul1
MCHUNK = 4096            # M load chunk (SBUF M tile free size)
MCH = N // MCHUNK        # 4
NCHUNK = 2048            # output store chunk (SBUF out tile free size)
NCH = N // NCHUNK        # 8

# matmul dtypes ("float32" = exact 2-pass fp32; "float32r" = single-pass
# fp32; "bfloat16" = half the HBM bytes; "float8e4" = quarter bytes).
# fp8 tensors are pre-scaled by 32 on the host (their raw values sit in
# e4m3's subnormal range) and the inverse scale is folded into the mm1
# eviction activation, so mm2's eviction stays a plain DC-add.  hT stays
# bf16 (mixed bf16 x fp8 matmuls run at full rate).
MM1_DT = "float8e4"          # x, w1t, w0
MM2_DT = "float8e4"          # combined IDFT-basis + Ws.T matrix
FP8_SCALE = 32.0
# fp8 DoubleRow perf mode: 2 contraction sub-tiles per matmul pass
USE_DR = True

_NC_CACHE = {}


def _scale_of(name):
    return FP8_SCALE if name == "float8e4" else 1.0


def _build_nc(mm1_name, mm2_name):
    import concourse.bacc as bacc
    import concourse.mybir as mybir
    import concourse.tile as tile

    f32 = mybir.dt.float32
    mm1 = getattr(mybir.dt, mm1_name)
    mm2 = getattr(mybir.dt, mm2_name)
    # hT feeds mm2 as lhsT; bf16 when M is narrow (mixed bf16 x fp8 matmuls
    # run at full PE rate -- but interleaving fp32 matmuls into the mm2
    # stream forces pipeline drains, so the dc transpose stays out of it)
    hdt = mybir.dt.bfloat16 if mm2_name in ("bfloat16", "float8e4") else mm2
    s1 = _scale_of(mm1_name)
    s2 = _scale_of(mm2_name)
    dr = USE_DR and mm1_name == "float8e4"
    DR = mybir.MatmulPerfMode.DoubleRow if dr else None

    nc = bacc.Bacc("TRN2", target_bir_lowering=False)

    xT = nc.dram_tensor("xT", [P, KT, BC], mm1, kind="ExternalInput")
    w1t = nc.dram_tensor("w1t", [P, KT, P], mm1, kind="ExternalInput")
    w0 = nc.dram_tensor("w0", [P, KT, 2], mm1, kind="ExternalInput")
    mmat = nc.dram_tensor("mmat", [P, N], mm2, kind="ExternalInput")
    # btc col 0: b_proj[1:129]/s2 bias; col 1: DC const b0/sqrt(N), all rows
    btc = nc.dram_tensor("btc", [P, 2], f32, kind="ExternalInput")
    out = nc.dram_tensor("out", [BC, N], f32, kind="ExternalOutput")

    Ident = mybir.ActivationFunctionType.Identity

    with tile.TileContext(nc) as tc:
        with (
            tc.tile_pool(name="wp", bufs=1) as wp,
            tc.tile_pool(name="xp", bufs=1) as xp,
            tc.tile_pool(name="mp", bufs=1) as mp,
            tc.tile_pool(name="hp", bufs=1) as hp,
            tc.tile_pool(name="op", bufs=4) as op,
            tc.tile_pool(name="ps", bufs=5, space="PSUM") as ps,
            tc.tile_pool(name="ps1", bufs=1, space="PSUM") as ps1,
            tc.tile_pool(name="ps2", bufs=1, space="PSUM") as ps2,
        ):
            # loads dispatch from the Sync engine in dependency order (each
            # dma_start costs ~650ns of serial dispatch there); stores also
            # go on Sync -- it is idle once the load dispatches are done
            # x first (the long pole), one DMA: 8KB contiguous rows hit
            # the full per-queue rate; w1t lands while mm1's first ldweights
            # would still be waiting on x anyway
            x_sb = xp.tile([P, KT, BC], mm1, tag="x")
            nc.sync.dma_start(out=x_sb[:, :, :], in_=xT[:, :, :])
            w1t_sb = wp.tile([P, KT, P], mm1, tag="w1t")
            nc.sync.dma_start(out=w1t_sb[:, :, :], in_=w1t[:, :, :])

            # tiny tensors next: the dc chain needs them right after mm1
            w0_sb = wp.tile([P, KT, 2], mm1, tag="w0")
            nc.sync.dma_start(out=w0_sb[:, :, :], in_=w0[:, :, :])
            btc_sb = wp.tile([P, 2], f32, tag="btc")
            nc.sync.dma_start(out=btc_sb[:, :], in_=btc[:, :])
            ones_sb = wp.tile([1, 1], mybir.dt.bfloat16, tag="ones")
            nc.vector.memset(ones_sb[:, :], 1.0)

            # combined IDFT-basis + Ws.T matrix, resident in SBUF
            mm = []
            for ti in range(MCH):
                m = mp.tile([P, MCHUNK], mm2, tag=f"m{ti}")
                nc.sync.dma_start(
                    out=m[:, :], in_=mmat[:, ti * MCHUNK:(ti + 1) * MCHUNK]
                )
                mm.append(m)

            # matmul1: hT[d, b] for d = h dims 1..128 (fp8 DoubleRow: two
            # contraction sub-tiles per pass)
            kstep = 2 if dr else 1
            hT_ps = ps1.tile([P, BC], f32, tag="hT")
            for kt in range(0, KT, kstep):
                nc.tensor.matmul(
                    hT_ps[:, :],
                    lhsT=w1t_sb[:, kt:kt + kstep, :],
                    rhs=x_sb[:, kt:kt + kstep, :],
                    start=(kt == 0),
                    stop=(kt + kstep == KT),
                    perf_mode=DR,
                )
            # evict h/(s1*s2): 1/s1 undoes the host fp8 pre-scale of w1t, the
            # extra 1/s2 cancels against M's host pre-scale in mm2.  btc col0
            # is pre-divided by s2 on the host.
            hT_sb = hp.tile([P, BC], hdt, tag="hT_sb")
            nc.scalar.activation(
                hT_sb[:, :], hT_ps[:, :], Ident,
                bias=btc_sb[:, 0:1], scale=float(1.0 / (s1 * s2)),
            )

            # dc row: h dim 0 as (1, BC), scaled to the final per-batch
            # output bias, then PE-transposed to (P, BC//P)
            dcr_ps = ps2.tile([1, BC], f32, tag="dcr")
            for kt in range(KT):
                nc.tensor.matmul(
                    dcr_ps[:, :],
                    lhsT=w0_sb[:, kt:kt + 1, 0:1],
                    rhs=x_sb[:, kt:kt + 1, :],
                    start=(kt == 0),
                    stop=(kt == KT - 1),
                )
            # bf16 here: an fp32 lhsT in the transpose matmuls would force
            # LOW/HIGH ldweights pairs into the PE stream (costs ~1.4e-3 rel)
            dcr_sb = hp.tile([1, BC], mybir.dt.bfloat16, tag="dcr_sb")
            nc.scalar.activation(
                dcr_sb[:, :], dcr_ps[0:1, :], Ident,
                bias=btc_sb[0:1, 1:2], scale=float(1.0 / (s1 * np.sqrt(N))),
            )
            dc_sb = hp.tile([P, BC // P], f32, tag="dc_sb")
            for j in range(BC // P):
                dcc_ps = ps2.tile([P, 1], f32, tag="dcc")
                nc.tensor.matmul(
                    dcc_ps[:, :],
                    lhsT=dcr_sb[0:1, j * P:(j + 1) * P],
                    rhs=ones_sb[0:1, 0:1],
                    start=True,
                    stop=True,
                )
                nc.scalar.copy(dc_sb[:, j:j + 1], dcc_ps[:, :])

            # matmul2 + DC bias-add eviction (split scalar/vector) + store.
            # The first and last output blocks store as 512-col mini-tiles:
            # the first store fires after a single eviction and the final
            # drain shrinks 4x.
            ev = 0
            for ti in range(NCH):
                mt = mm[ti * NCHUNK // MCHUNK]
                mo = (ti * NCHUNK) % MCHUNK
                for j in range(BC // P):
                    mini = (ti == 0 and j == 0) or (ti == NCH - 1 and j == BC // P - 1)
                    nsub = NCHUNK // 512
                    ob = None if mini else op.tile([P, NCHUNK], f32, tag="ob")
                    for s in range(nsub):
                        pt = ps.tile([P, 512], f32, tag="mm2")
                        nc.tensor.matmul(
                            pt[:, :],
                            lhsT=hT_sb[:, j * P:(j + 1) * P],
                            rhs=mt[:, mo + s * 512:mo + (s + 1) * 512],
                            start=True,
                            stop=True,
                        )
                        if mini:
                            dst = op.tile([P, 512], f32, tag="obm")
                        else:
                            dst = ob[:, s * 512:(s + 1) * 512]
                        if ev % 2 == 0:
                            nc.vector.tensor_scalar_add(
                                dst, pt[:, :], dc_sb[:, j:j + 1]
                            )
                        else:
                            nc.scalar.activation(
                                dst, pt[:, :], Ident, bias=dc_sb[:, j:j + 1],
                            )
                        ev += 1
                        if mini:
                            nc.sync.dma_start(
                                out=out[j * P:(j + 1) * P,
                                        ti * NCHUNK + s * 512:ti * NCHUNK + (s + 1) * 512],
                                in_=dst,
                            )
                    if not mini:
                        nc.sync.dma_start(
                            out=out[j * P:(j + 1) * P, ti * NCHUNK:(ti + 1) * NCHUNK],
                            in_=ob[:, :],
                        )
    nc.compile()
    return nc


def _get_nc():
    key = (MM1_DT, MM2_DT, USE_DR)
    if key not in _NC_CACHE:
        _NC_CACHE[key] = _build_nc(MM1_DT, MM2_DT)
    return _NC_CACHE[key]


def _np_dt(name):
    if name.startswith("float32"):
        return np.float32
    import ml_dtypes
    if name == "float8e4":
        return np.dtype(ml_dtypes.float8_e4m3fn)
    return np.dtype(getattr(ml_dtypes, name))


def _host_pack(x, W_proj, b_proj, Ws):
    dt1 = _np_dt(MM1_DT)
    dt2 = _np_dt(MM2_DT)
    s1 = _scale_of(MM1_DT)
    s2 = _scale_of(MM2_DT)
    SQRT2 = np.float64(np.sqrt(np.float32(2.0)))
    n_idx = np.arange(N, dtype=np.float64)
    k_idx = np.arange(1, K + 1, dtype=np.float64)
    theta = (2.0 * np.pi / N) * np.outer(k_idx, n_idx)
    M = np.empty((P, N), np.float32)
    isqn = 1.0 / np.sqrt(np.float64(N))
    M[0:2 * K:2] = (SQRT2 * isqn) * np.cos(theta)
    M[1:2 * K:2] = (SQRT2 * isqn) * np.sin(theta)
    M[2 * K:] = Ws.T
    M = np.ascontiguousarray(M * np.float32(s2), dt2)

    w1 = W_proj[1:P + 1] * np.float32(s1)                 # (128, 2048)
    w1t = w1.T.reshape(KT, P, P).transpose(1, 0, 2)       # (128, 16, 128)
    w0 = np.ascontiguousarray(
        np.repeat(
            (W_proj[0].reshape(KT, P).T * np.float32(s1))[:, :, None], 2, axis=2
        ),
        dt1,
    )                                                     # (128, 16, 2)
    btc = np.zeros((P, 2), np.float32)
    btc[:, 0] = b_proj[1:P + 1] / np.float32(s2)
    btc[:, 1] = np.float32(np.float64(b_proj[0]) / np.sqrt(np.float64(N)))

    w1t = np.ascontiguousarray(w1t, dt1)
    xts = []
    for c in range(NCORES):
        xc = x[c * BC:(c + 1) * BC]                        # (512, 2048)
        xt = np.ascontiguousarray(
            xc.T.reshape(KT, P, BC).transpose(1, 0, 2), dt1
        )                                                 # (128, 16, 512)
        xts.append(xt)
    return M, w1t, w0, btc, xts


def kernel(x, W_proj, b_proj, Ws, _trace=False, _tmpdir=None):
    from concourse import bass_utils

    x = np.ascontiguousarray(x, np.float32)
    W_proj = np.ascontiguousarray(W_proj, np.float32)
    b_proj = np.ascontiguousarray(b_proj, np.float32)
    Ws = np.ascontiguousarray(Ws, np.float32)

    M, w1t, w0, btc, xts = _host_pack(x, W_proj, b_proj, Ws)
    nc = _get_nc()

    in_maps = [
        {"xT": xts[c], "w1t": w1t, "w0": w0, "mmat": M, "btc": btc}
        for c in range(NCORES)
    ]
    kw = {}
    if _trace:
        kw = dict(trace=True, tmpdir=_tmpdir, trace_cores=[0])
    res = bass_utils.run_bass_kernel_spmd(nc, in_maps, core_ids=list(range(NCORES)), **kw)
    out = np.concatenate([r["out"] for r in res.results], axis=0)
    if _trace:
        return out, res
    return out



# revision 49
# speedup vs baseline: 1.1109x; 1.0156x over previous
"""KSparseFFTClassifier Trainium2 kernel.

Math: reference computes
    h   = x @ W_proj.T + b_proj                      (bs, 129)
    h  *= scale  (sqrt(2) on dims 1..64)
    out = IDFT65(h[:, :65]) + h[:, 65:] @ Ws.T       (bs, 16384)

The zero-padded orthonormal IDFT of the 65 nonzero frequency components is a
dense matmul against a (65, N) cos/sin basis; the DC row of that basis is the
constant 1/sqrt(N).  So with M = [scaled cos/sin basis for h dims 1..64;
Ws.T]  (128 x N):

    out[b, n] = h[b, 1:129] @ M[:, n] + (h[b, 0] + 0) / sqrt(N)

i.e. a (bs,2048)x(2048,128) matmul, a (bs,128)x(128,N) matmul, and a
per-row scalar (the DC term) added during PSUM eviction.

Sharding: data-parallel over batch, 512 rows per core on 8 cores.
"""

import numpy as np

BS = 4096
IN_DIM = 2048
N = 16384
K = 32
SLACK = 64
NCORES = 8
BC = BS // NCORES        # 512 batch rows per core
P = 128
KT = IN_DIM // P         # 16 contraction tiles for matmul1
MCHUNK = 4096            # M load chunk (SBUF M tile free size)
MCH = N // MCHUNK        # 4
NCHUNK = 2048            # output store chunk (SBUF out tile free size)
NCH = N // NCHUNK        # 8

# matmul dtypes ("float32" = exact 2-pass fp32; "float32r" = single-pass
# fp32; "bfloat16" = half the HBM bytes; "float8e4" = quarter bytes).
# fp8 tensors are pre-scaled by 32 on the host (their raw values sit in
# e4m3's subnormal range) and the inverse scale is folded into the mm1
# eviction activation, so mm2's eviction stays a plain DC-add.  hT stays
# bf16 (mixed bf16 x fp8 matmuls run at full rate).
MM1_DT = "float8e4"          # x, w1t, w0
MM2_DT = "float8e4"          # combined IDFT-basis + Ws.T matrix
FP8_SCALE = 32.0
# fp8 DoubleRow perf mode: 2 contraction sub-tiles per matmul pass
USE_DR = True

_NC_CACHE = {}


def _scale_of(name):
    return FP8_SCALE if name == "float8e4" else 1.0


def _build_nc(mm1_name, mm2_name):
    import concourse.bacc as bacc
    import concourse.mybir as mybir
    import concourse.tile as tile

    f32 = mybir.dt.float32
    mm1 = getattr(mybir.dt, mm1_name)
    mm2 = getattr(mybir.dt, mm2_name)
    # hT feeds mm2 as lhsT; bf16 when M is narrow (mixed bf16 x fp8 matmuls
    # run at full PE rate -- but interleaving fp32 matmuls into the mm2
    # stream forces pipeline drains, so the dc transpose stays out of it)
    hdt = mybir.dt.bfloat16 if mm2_name in ("bfloat16", "float8e4") else mm2
    s1 = _scale_of(mm1_name)
    s2 = _scale_of(mm2_name)
    dr = USE_DR and mm1_name == "float8e4"
    DR = mybir.MatmulPerfMode.DoubleRow if dr else None

    nc = bacc.Bacc("TRN2", target_bir_lowering=False)

    JB = BC // P
    XP = P + 16  # inner pad: keeps the DoubleRow kt-pair AP dims unmergeable
    # x j-major: per batch j-block, k-major blocks of P columns
    xT = nc.dram_tensor("xT", [P, JB * KT, XP], mm1, kind="ExternalInput")
    w1t = nc.dram_tensor("w1t", [P, KT, P], mm1, kind="ExternalInput")
    w0 = nc.dram_tensor("w0", [P, KT, 2], mm1, kind="ExternalInput")
    mmat = nc.dram_tensor("mmat", [P, N], mm2, kind="ExternalInput")
    # btc col 0: b_proj[1:129]/s2 bias; col 1: DC const b0/sqrt(N), all rows
    btc = nc.dram_tensor("btc", [P, 2], f32, kind="ExternalInput")
    out = nc.dram_tensor("out", [BC, N], f32, kind="ExternalOutput")

    Ident = mybir.ActivationFunctionType.Identity

    with tile.TileContext(nc) as tc:
        with (
            tc.tile_pool(name="wp", bufs=1) as wp,
            tc.tile_pool(name="xp", bufs=1) as xp,
            tc.tile_pool(name="mp", bufs=1) as mp,
            tc.tile_pool(name="hp", bufs=1) as hp,
            tc.tile_pool(name="op", bufs=4) as op,
            tc.tile_pool(name="ps", bufs=5, space="PSUM") as ps,
            tc.tile_pool(name="ps1", bufs=1, space="PSUM") as ps1,
            tc.tile_pool(name="ps2", bufs=1, space="PSUM") as ps2,
        ):
            # loads dispatch from the Sync engine (each dma_start costs
            # ~650ns of serial dispatch there).  The whole kernel is
            # pipelined per batch j-block: as soon as x's first j-block
            # lands, mm1/dcr/mm2 run for it and its stores start while the
            # remaining j-blocks are still loading.
            w1t_sb = wp.tile([P, KT, P], mm1, tag="w1t")
            nc.sync.dma_start(out=w1t_sb[:, :, :], in_=w1t[:, :, :])
            x_sb = xp.tile([P, JB * KT, XP], mm1, tag="x")
            nc.sync.dma_start(out=x_sb[:, 0:KT, :], in_=xT[:, 0:KT, :])
            w0_sb = wp.tile([P, KT, 2], mm1, tag="w0")
            nc.sync.dma_start(out=w0_sb[:, :, :], in_=w0[:, :, :])
            btc_sb = wp.tile([P, 2], f32, tag="btc")
            nc.sync.dma_start(out=btc_sb[:, :], in_=btc[:, :])
            ones_sb = wp.tile([1, 1], mybir.dt.bfloat16, tag="ones")
            nc.vector.memset(ones_sb[:, :], 1.0)

            # combined IDFT-basis + Ws.T matrix, resident in SBUF
            mm = []
            for ti in range(MCH):
                m = mp.tile([P, MCHUNK], mm2, tag=f"m{ti}")
                mm.append(m)
            nc.sync.dma_start(out=mm[0][:, :], in_=mmat[:, 0:MCHUNK])
            for j in range(1, JB):
                nc.sync.dma_start(
                    out=x_sb[:, j * KT:(j + 1) * KT, :],
                    in_=xT[:, j * KT:(j + 1) * KT, :],
                )
            for ti in range(1, MCH):
                nc.sync.dma_start(
                    out=mm[ti][:, :], in_=mmat[:, ti * MCHUNK:(ti + 1) * MCHUNK]
                )

            kstep = 2 if dr else 1
            hT_ps = ps1.tile([P, BC], f32, tag="hT")
            hT_sb = hp.tile([P, BC], hdt, tag="hT_sb")
            dcr_ps = ps2.tile([1, BC], f32, tag="dcr")
            dcr_sb = hp.tile([1, BC], mybir.dt.bfloat16, tag="dcr_sb")
            dc_sb = hp.tile([P, JB], f32, tag="dc_sb")

            ev = 0
            for j in range(JB):
                jc = slice(j * P, (j + 1) * P)
                # matmul1 for this j-block (fp8 DoubleRow: two contraction
                # sub-tiles per pass)
                for kt in range(0, KT, kstep):
                    nc.tensor.matmul(
                        hT_ps[:, jc],
                        lhsT=w1t_sb[:, kt:kt + kstep, :],
                        rhs=x_sb[:, j * KT + kt:j * KT + kt + kstep, 0:P],
                        start=(kt == 0),
                        stop=(kt + kstep == KT),
                        perf_mode=DR,
                    )
                # evict h/(s1*s2): 1/s1 undoes the host fp8 pre-scale of
                # w1t, the extra 1/s2 cancels against M's host pre-scale in
                # mm2.  btc col0 is pre-divided by s2 on the host.
                nc.scalar.activation(
                    hT_sb[:, jc], hT_ps[:, jc], Ident,
                    bias=btc_sb[:, 0:1], scale=float(1.0 / (s1 * s2)),
                )

                # dc row piece: h dim 0 for this j-block, scaled to the
                # final per-batch output bias, then PE-transposed to (P, 1).
                # bf16 dcr: an fp32 lhsT in the transpose matmul would force
                # LOW/HIGH ldweights pairs into the PE stream.
                for kt in range(KT):
                    nc.tensor.matmul(
                        dcr_ps[0:1, jc],
                        lhsT=w0_sb[:, kt:kt + 1, 0:1],
                        rhs=x_sb[:, j * KT + kt:j * KT + kt + 1, 0:P],
                        start=(kt == 0),
                        stop=(kt == KT - 1),
                    )
                nc.scalar.activation(
                    dcr_sb[0:1, jc], dcr_ps[0:1, jc], Ident,
                    bias=btc_sb[0:1, 1:2], scale=float(1.0 / (s1 * np.sqrt(N))),
                )
                dcc_ps = ps2.tile([P, 1], f32, tag="dcc")
                nc.tensor.matmul(
                    dcc_ps[:, :],
                    lhsT=dcr_sb[0:1, jc],
                    rhs=ones_sb[0:1, 0:1],
                    start=True,
                    stop=True,
                )
                nc.scalar.copy(dc_sb[:, j:j + 1], dcc_ps[:, :])

                # matmul2 + DC bias-add eviction (split scalar/vector) +
                # store, for every output chunk of this j-block.  The very
                # first and last blocks store as 512-col mini-tiles: the
                # first store fires after a single eviction and the final
                # drain shrinks 4x.
                for ti in range(NCH):
                    mt = mm[ti * NCHUNK // MCHUNK]
                    mo = (ti * NCHUNK) % MCHUNK
                    mini = (ti == 0 and j == 0) or (ti == NCH - 1 and j == JB - 1)
                    ob = None if mini else op.tile([P, NCHUNK], f32, tag="ob")
                    for s in range(NCHUNK // 512):
                        pt = ps.tile([P, 512], f32, tag="mm2")
                        nc.tensor.matmul(
                            pt[:, :],
                            lhsT=hT_sb[:, jc],
                            rhs=mt[:, mo + s * 512:mo + (s + 1) * 512],
                            start=True,
                            stop=True,
                        )
                        if mini:
                            dst = op.tile([P, 512], f32, tag="obm")
                        else:
                            dst = ob[:, s * 512:(s + 1) * 512]
                        if ev % 2 == 0:
                            nc.vector.tensor_scalar_add(
                                dst, pt[:, :], dc_sb[:, j:j + 1]
                            )
                        else:
                            nc.scalar.activation(
                                dst, pt[:, :], Ident, bias=dc_sb[:, j:j + 1],
                            )
                        ev += 1
                        if mini:
                            nc.sync.dma_start(
                                out=out[jc,
                                        ti * NCHUNK + s * 512:ti * NCHUNK + (s + 1) * 512],
                                in_=dst,
                            )
                    if not mini:
                        nc.sync.dma_start(
                            out=out[jc, ti * NCHUNK:(ti + 1) * NCHUNK],
                            in_=ob[:, :],
                        )
    nc.compile()
    return nc


def _get_nc():
    key = (MM1_DT, MM2_DT, USE_DR)
    if key not in _NC_CACHE:
        _NC_CACHE[key] = _build_nc(MM1_DT, MM2_DT)
    return _NC_CACHE[key]


def _np_dt(name):
    if name.startswith("float32"):
        return np.float32
    import ml_dtypes
    if name == "float8e4":
        return np.dtype(ml_dtypes.float8_e4m3fn)
    return np.dtype(getattr(ml_dtypes, name))


def _host_pack(x, W_proj, b_proj, Ws):
    dt1 = _np_dt(MM1_DT)
    dt2 = _np_dt(MM2_DT)
    s1 = _scale_of(MM1_DT)
    s2 = _scale_of(MM2_DT)
    SQRT2 = np.float64(np.sqrt(np.float32(2.0)))
    n_idx = np.arange(N, dtype=np.float64)
    k_idx = np.arange(1, K + 1, dtype=np.float64)
    theta = (2.0 * np.pi / N) * np.outer(k_idx, n_idx)
    M = np.empty((P, N), np.float32)
    isqn = 1.0 / np.sqrt(np.float64(N))
    M[0:2 * K:2] = (SQRT2 * isqn) * np.cos(theta)
    M[1:2 * K:2] = (SQRT2 * isqn) * np.sin(theta)
    M[2 * K:] = Ws.T
    M = np.ascontiguousarray(M * np.float32(s2), dt2)

    w1 = W_proj[1:P + 1] * np.float32(s1)                 # (128, 2048)
    w1t = w1.T.reshape(KT, P, P).transpose(1, 0, 2)       # (128, 16, 128)
    w0 = np.ascontiguousarray(
        np.repeat(
            (W_proj[0].reshape(KT, P).T * np.float32(s1))[:, :, None], 2, axis=2
        ),
        dt1,
    )                                                     # (128, 16, 2)
    btc = np.zeros((P, 2), np.float32)
    btc[:, 0] = b_proj[1:P + 1] / np.float32(s2)
    btc[:, 1] = np.float32(np.float64(b_proj[0]) / np.sqrt(np.float64(N)))

    w1t = np.ascontiguousarray(w1t, dt1)
    xts = []
    for c in range(NCORES):
        xc = x[c * BC:(c + 1) * BC]                        # (512, 2048)
        xt = xc.T.reshape(KT, P, BC // P, P).transpose(1, 2, 0, 3)
        xtp = np.zeros((P, BC // P, KT, P + 16), np.float32)
        xtp[:, :, :, :P] = xt
        xts.append(np.ascontiguousarray(xtp, dt1))        # (128, 4, 16, 144)
    return M, w1t, w0, btc, xts


def kernel(x, W_proj, b_proj, Ws, _trace=False, _tmpdir=None):
    from concourse import bass_utils

    x = np.ascontiguousarray(x, np.float32)
    W_proj = np.ascontiguousarray(W_proj, np.float32)
    b_proj = np.ascontiguousarray(b_proj, np.float32)
    Ws = np.ascontiguousarray(Ws, np.float32)

    M, w1t, w0, btc, xts = _host_pack(x, W_proj, b_proj, Ws)
    nc = _get_nc()

    in_maps = [
        {"xT": xts[c], "w1t": w1t, "w0": w0, "mmat": M, "btc": btc}
        for c in range(NCORES)
    ]
    kw = {}
    if _trace:
        kw = dict(trace=True, tmpdir=_tmpdir, trace_cores=[0])
    res = bass_utils.run_bass_kernel_spmd(nc, in_maps, core_ids=list(range(NCORES)), **kw)
    out = np.concatenate([r["out"] for r in res.results], axis=0)
    if _trace:
        return out, res
    return out

